# revision 1
# baseline (speedup 1.0000x reference)
"""Multi-head self-attention (16 heads, hd=64, RoPE, causal) on 8 trn2 cores.

Sharding: DP(batch=2) x TP(head-groups=4). Core c handles batch c//4, heads
[4*(c%4), 4*(c%4)+4). Each core computes a row-parallel partial output
yT_partial [1024, 2048]; host sums the 4 partials per batch and transposes.
No device-device communication.

Device kernel works in transposed layout throughout:
  - xT [e, t] streamed from DRAM
  - qT/kT [d_local, t] with per-head de-interleaved RoPE halves (weight rows
    pre-permuted on host so rot1/rot2 operate on contiguous 32-row blocks)
  - scoresT [kt, qt] per head; K=64 matmuls pair-packed via tile_position
  - probs = exp(scoresT) (no max subtraction; scores are O(1) by construction)
  - attnT [vd, qt] = v_aug.T @ probs with M=65 (65th column of v_aug is ones,
    yielding the softmax denominator row for free)
  - yT [e, qt] = woT.T @ attnT_normalized
"""

import sys

for _p in ("/opt/trn_rl_repo",):
    if _p not in sys.path:
        sys.path.insert(0, _p)

import numpy as np

import concourse.bass as bass
import concourse.mybir as mybir
import concourse.tile as tile
from concourse import bacc
from concourse.bass_utils import run_bass_kernel_spmd

F32 = mybir.dt.float32
F32R = mybir.dt.float32r
BF16 = mybir.dt.bfloat16
AF = mybir.ActivationFunctionType

# matmul groups run in float32r (1 cyc/row at N>=256 vs 4 for fp32).
FP32R_GROUPS = {"qkv", "scores", "attnv", "yt", "bcast"}


def _mm(group, ap):
    return ap.bitcast(F32R) if group in FP32R_GROUPS else ap

B, T, E = 2, 2048, 1024
NH, HD = 16, 64
NHL = 4          # heads per core
DL = NHL * HD    # 256 local head dims
NCORES = 8
NEG = -1e9
ROPE_BASE = 10000.0


# ----------------------------------------------------------------- device IR
def build_module(reps=1):
    nc = bacc.Bacc("TRN2", target_bir_lowering=False, debug=False,
                   num_devices=NCORES)

    xt = nc.dram_tensor("xt", [E, T], F32R, kind="ExternalInput").ap()
    wqt = nc.dram_tensor("wqt", [E, DL], F32R, kind="ExternalInput").ap()
    wkt = nc.dram_tensor("wkt", [E, DL], F32R, kind="ExternalInput").ap()
    wvt = nc.dram_tensor("wvt", [E, DL], F32R, kind="ExternalInput").ap()
    wot = nc.dram_tensor("wot", [DL, E], F32R, kind="ExternalInput").ap()
    cd = nc.dram_tensor("cd", [128, T], F32R, kind="ExternalInput").ap()
    sd = nc.dram_tensor("sd", [128, T], F32R, kind="ExternalInput").ap()
    negid = nc.dram_tensor("negid", [128, 128], BF16, kind="ExternalInput").ap()
    stepd = nc.dram_tensor("stepd", [128, 128], BF16, kind="ExternalInput").ap()
    onesd = nc.dram_tensor("onesd", [128, 64], F32R, kind="ExternalInput").ap()
    yt = nc.dram_tensor("yt", [E, T], F32, kind="ExternalOutput").ap()

    with tile.TileContext(nc) as tc:
        for _ in range(reps):
            _body(tc, xt, wqt, wkt, wvt, wot, cd, sd, negid, stepd, onesd, yt)
    nc.compile()
    return nc


def _chunks(qs_rel):
    """512-bank-aligned column chunks of [qs_rel, 1024)."""
    out = []
    if qs_rel < 512:
        out.append((qs_rel, 512))
        out.append((512, 1024))
    else:
        out.append((qs_rel, 1024))
    return out


def _body(tc, xt, wqt, wkt, wvt, wot, cd, sd, negid, stepd, onesd, yt):
    nc = tc.nc
    from contextlib import ExitStack

    with ExitStack() as outer:
        # all SBUF pools at one level: no pool-close gates between phases
        po = outer.enter_context(tc.tile_pool(name="persist", bufs=1))
        wp = outer.enter_context(tc.tile_pool(name="wp", bufs=1))
        xcp = outer.enter_context(tc.tile_pool(name="xcp", bufs=10))
        swpp = outer.enter_context(tc.tile_pool(name="swp", bufs=2))
        expp = outer.enter_context(tc.tile_pool(name="expp", bufs=4))
        dnp = outer.enter_context(tc.tile_pool(name="dnp", bufs=2))
        yp = outer.enter_context(tc.tile_pool(name="yp", bufs=4))

        # persistent tiles
        qk = {}
        for nm in ("q0", "q1", "k0", "k1"):
            qk[nm] = po.tile([128, T], F32R, tag=nm, name=nm)
        v_sb = po.tile([128, 16 * 260], F32R, tag="v")
        wot_sb = [po.tile([128, E], F32R, tag=f"wot{p}", name=f"wot{p}")
                  for p in range(2)]
        at = [po.tile([128, T], F32R, tag=f"at{p}", name=f"at{p}")
              for p in range(2)]
        negi_sb = po.tile([128, 128], BF16, tag="negi")
        step_sb = po.tile([128, 128], BF16, tag="step")
        ones_sb = po.tile([65, 64], F32R, tag="ones")

        w_sb = {}
        w_srcs = {"wq": wqt, "wk": wkt, "wv": wvt}
        for nm in ("wq", "wk", "wv"):
            w_sb[nm] = wp.tile([128, 2048], F32R, tag=nm, name=nm)

        def _wload(nm):
            nc.sync.dma_start(
                out=w_sb[nm][:].rearrange("p (eo d) -> p eo d", eo=8),
                in_=w_srcs[nm].rearrange("(eo p) d -> p eo d", p=128))

        _wload("wq")
        _wload("wk")
        _wload("wv")
        trig = {}
        for nm in ("c", "s"):
            trig[nm] = wp.tile([128, T], F32R, tag=nm, name="trig_" + nm)

        def _late_loads():
            # issued after the first window's x chunks so the DMA queues
            # prioritize what phase 1 needs first
            nc.sync.dma_start(out=trig["c"][:], in_=cd[:])
            nc.sync.dma_start(out=trig["s"][:], in_=sd[:])
            nc.sync.dma_start(out=negi_sb[:], in_=negid[:])
            nc.sync.dma_start(out=step_sb[:], in_=stepd[:])
            for p in range(2):
                nc.sync.dma_start(out=wot_sb[p][:],
                                  in_=wot[p * 128:(p + 1) * 128, :])
            nc.sync.dma_start(out=ones_sb[64:65, :], in_=onesd[0:1, 0:64])
            v_ones_view = v_sb[:].rearrange("p (tt h x) -> p tt h x",
                                            tt=16, h=4)
            nc.sync.dma_start(
                out=v_ones_view[:, :, :, 64:65],
                in_=onesd[:, 0:64].rearrange("p (tt h) -> p tt h",
                                             tt=16)[:, :, :, None])

        # ---------------- phase 1: projections + rope -----------------------
        with tc.tile_pool(name="pp", bufs=1, space="PSUM") as pp:
            for tcx in range(2):
                for half in range(2):
                    c0 = half * 512
                    tw = tcx * 1024 + c0
                    xc = []
                    for eo in range(8):
                        t_ = xcp.tile([128, 512], F32R, tag="xc", name="xc")
                        eng = nc.sync
                        eng.dma_start(
                            out=t_[:],
                            in_=xt[eo * 128:(eo + 1) * 128, tw:tw + 512])
                        xc.append(t_)
                    if tcx == 0 and half == 0:
                        _late_loads()
                    ps = {nm: pp.tile([128, 512], F32, tag="qkps", bufs=4,
                                      name="ps_" + nm)
                          for nm in ("q0", "q1", "k0", "k1")}
                    for eo in range(8):
                        for wnm, dh in (("wq", 0), ("wq", 1),
                                        ("wk", 0), ("wk", 1)):
                            dst = ("q" if wnm == "wq" else "k") + str(dh)
                            nc.tensor.matmul(
                                out=ps[dst][:],
                                lhsT=w_sb[wnm][:, eo * 256 + dh * 128:
                                               eo * 256 + dh * 128 + 128],
                                rhs=xc[eo][:],
                                start=(eo == 0), stop=(eo == 7))
                    for i_, nm in enumerate(("q0", "q1", "k0", "k1")):
                        if i_ % 2 == 0:
                            nc.vector.tensor_copy(
                                qk[nm][:, tw:tw + 512], ps[nm][:])
                        else:
                            nc.scalar.copy(
                                qk[nm][:, tw:tw + 512], ps[nm][:])
                    # V: tt-outer, eo-inner so only 2 psum banks needed
                    for tt_ in range(4):
                        psv = pp.tile([128, 256], F32, tag="vps", bufs=4,
                                      name="psv")
                        for eo in range(8):
                            nc.tensor.matmul(
                                out=psv[:],
                                lhsT=xc[eo][:, tt_ * 128:tt_ * 128 + 128],
                                rhs=w_sb["wv"][:, eo * 256:(eo + 1) * 256],
                                start=(eo == 0), stop=(eo == 7))
                        gt = tw // 128 + tt_
                        dst = v_sb[:, gt * 260:(gt + 1) * 260] \
                            .rearrange("p (h x) -> p h x", h=4)
                        src_ = psv[:].rearrange("p (h x) -> p h x", h=4)
                        if tt_ % 2 == 0:
                            nc.vector.tensor_copy(dst[:, :, 0:64],
                                                  src_[:, :, 0:64])
                        else:
                            nc.scalar.copy(dst[:, :, 0:64],
                                           src_[:, :, 0:64])
                # rope this 1024-window right after its projections
                # (pair-0 tiles first so attention can start early)
                for nm in ("q0", "k0", "q1", "k1"):
                    cs = slice(tcx * 1024, tcx * 1024 + 1024)
                    raw = qk[nm]
                    swp = swpp.tile([128, 1024], F32R, tag="swp")
                    for j in range(4):
                        a, b_ = j * 32, (j ^ 1) * 32
                        nc.sync.dma_start(out=swp[a:a + 32, :],
                                          in_=raw[b_:b_ + 32, cs])
                    nc.vector.tensor_mul(raw[:, cs], raw[:, cs],
                                         trig["c"][:, cs])
                    nc.vector.tensor_mul(swp[:], swp[:], trig["s"][:, cs])
                    nc.vector.tensor_add(raw[:, cs], raw[:, cs], swp[:])

        # ---------------- phase 2 + 3: attention, yT interleaved -----------
        with tc.tile_pool(name="ps_s", bufs=2, space="PSUM") as ps_sp, \
             tc.tile_pool(name="ps_a", bufs=2, space="PSUM") as ps_ap:
            recip_pool = dnp
            ytv = yt.rearrange("(et p) t -> p et t", p=128)

            def yt_chunk(qc, pool, tag):
                # output projection for qt cols [qc*512, qc*512+512)
                for eg in range(2):
                    y_sb = yp.tile([128, 4 * 512], F32, tag="ysb", bufs=2,
                                   name="y_sb")
                    for ei in range(4):
                        et_ = eg * 4 + ei
                        ps_y = pool.tile([128, 512], F32, tag=tag,
                                         name="ps_y")
                        for p in range(2):
                            nc.tensor.matmul(
                                out=ps_y[:],
                                lhsT=wot_sb[p][:, et_ * 128:(et_ + 1) * 128],
                                rhs=at[p][:, qc * 512:(qc + 1) * 512],
                                start=(p == 0), stop=(p == 1))
                        if et_ % 2 == 0:
                            nc.vector.tensor_copy(
                                y_sb[:, ei * 512:(ei + 1) * 512], ps_y[:])
                        else:
                            nc.scalar.copy(
                                y_sb[:, ei * 512:(ei + 1) * 512], ps_y[:])
                    nc.sync.dma_start(
                        out=ytv[:, eg * 4:eg * 4 + 4,
                                qc * 512:(qc + 1) * 512],
                        in_=y_sb[:].rearrange("p (et t) -> p et t", et=4))

            def attention(pair, qtb):
                krot = [qk["k" + str(pair)], qk["k" + str(pair)]]
                qrot = [qk["q" + str(pair)], qk["q" + str(pair)]]
                nkt = 8 * qtb + 8
                q0 = qtb * 1024
                ps_a = [ps_ap.tile([128, 1024], F32, tag="a", name="ps_a")
                        for _ in range(2)]
                exp_tiles = [None] * nkt
                chunk_l = [None] * nkt

                def scores_step(kt):
                    qs_rel = max(0, 128 * kt - q0)
                    chs = _chunks(qs_rel)
                    chunk_l[kt] = chs
                    ps_s = [ps_sp.tile([128, 1024], F32, tag="s",
                                       name="ps_s")
                            for _ in range(2)]
                    diag = kt >= 8 * qtb
                    for h in range(2):
                        for (ca, cb) in chs:
                            is_diag_chunk = diag and ca == qs_rel
                            nc.tensor.matmul(
                                out=ps_s[h][:, ca:cb],
                                lhsT=krot[h][h * 64:h * 64 + 64,
                                             kt * 128:kt * 128 + 128],
                                rhs=qrot[h][h * 64:h * 64 + 64,
                                            q0 + ca:q0 + cb],
                                start=True, stop=not is_diag_chunk,
                                tile_position=(h * 64, 0))
                    if diag:
                        for h in range(2):
                            nc.tensor.matmul(
                                out=ps_s[h][:, qs_rel:qs_rel + 128],
                                lhsT=negi_sb[:],
                                rhs=step_sb[:],
                                start=False, stop=True)
                    et = []
                    for h in range(2):
                        e_ = expp.tile([128, 1024], F32R, tag="e",
                                       name="exp_t")
                        nc.scalar.activation(
                            e_[:, qs_rel:1024], ps_s[h][:, qs_rel:1024],
                            AF.Exp)
                        et.append(e_)
                    exp_tiles[kt] = et

                def attnv_step(kt):
                    # psum stop flags are bank-granular: bank0's last
                    # writer is kt=8*qtb+3, bank1's is nkt-1
                    et = exp_tiles[kt]
                    for h in range(2):
                        slot = kt * 260 + (2 * pair + h) * 65
                        for (ca, cb) in chunk_l[kt]:
                            last = 8 * qtb + 3 if ca < 512 else nkt - 1
                            nc.tensor.matmul(
                                out=ps_a[h][0:65, ca:cb],
                                lhsT=v_sb[:, slot:slot + 65],
                                rhs=et[h][:, ca:cb],
                                start=(kt == 0), stop=(kt == last))
                    exp_tiles[kt] = None

                for step in range(nkt + 1):
                    if step < nkt:
                        scores_step(step)
                    if step > 0:
                        attnv_step(step - 1)

                # normalization: denom rows staged in f32r dh tiles
                # (they feed the K=1 broadcast matmul); recip outputs in
                # plain f32 tiles (only consumed by DVE)
                dh0 = recip_pool.tile([128, 1024], F32R, tag="dh",
                                      name="dh0")
                dh1 = recip_pool.tile([128, 1024], F32R, tag="dh",
                                      name="dh1")
                nc.scalar.copy(dh0[64:65, :], ps_a[0][64:65, :])
                nc.scalar.copy(dh1[64:65, :], ps_a[1][64:65, :])
                ps_b = [ps_sp.tile([128, 1024], F32, tag="s",
                                   name="ps_b") for _ in range(2)]
                dns = (dh0, dh1)
                for h in range(2):
                    for (ca, cb) in ((0, 512), (512, 1024)):
                        nc.tensor.matmul(
                            out=ps_b[h][0:64, ca:cb],
                            lhsT=ones_sb[64:65, :],
                            rhs=dns[h][64:65, ca:cb],
                            start=True, stop=True,
                            tile_position=(64, 0))
                recip = recip_pool.tile([128, 1024], F32, tag="rc")
                scr = recip_pool.tile([128, 1024], F32, tag="scr")
                nc.vector.reciprocal_approx_fast(
                    out=recip[0:64, :], in_=ps_b[0][0:64, :])
                nc.vector.reciprocal_approx_fast(
                    out=scr[0:64, :], in_=ps_b[1][0:64, :])
                nc.vector.tensor_mul(
                    at[pair][0:64, q0:q0 + 1024],
                    ps_a[0][0:64, :], recip[0:64, :])
                a1n = recip_pool.tile([64, 1024], F32R, tag="dh",
                                      name="a1n")
                nc.vector.tensor_mul(
                    a1n[0:64, :], ps_a[1][0:64, :], scr[0:64, :])
                nc.sync.dma_start(
                    out=at[pair][64:128, q0:q0 + 1024], in_=a1n[0:64, :])

            for pair in range(2):
                for qtb in range(2):
                    attention(pair, qtb)

        # ---------------- phase 3: output projection ------------------------
        with tc.tile_pool(name="ps_y", bufs=4, space="PSUM") as ps_yp:
            for qc in range(4):
                yt_chunk(qc, ps_yp, "y")


# ----------------------------------------------------------------- host side
def _prep_core_inputs(x, wq, wk, wv, wo):
    """Build the 8 per-core input dicts (numpy fp32)."""
    # rope trig tables, transposed [freq, pos]
    inv_freq = 1.0 / (ROPE_BASE ** (np.arange(0, HD, 2, dtype=np.float32) / HD))
    pos = np.arange(T, dtype=np.float32)
    freqs = pos[:, None] * inv_freq[None, :]          # [T, 32]
    cosT = np.cos(freqs).T.astype(np.float32)          # [32, T]
    sinT = np.sin(freqs).T.astype(np.float32)
    C = np.tile(cosT, (4, 1)).astype(np.float32)       # [128, T]
    S = np.tile(np.concatenate([-sinT, sinT], axis=0), (2, 1)).astype(np.float32)
    scale = np.float32(1.0 / np.sqrt(HD))              # folded into wq

    import ml_dtypes
    r, c = np.indices((128, 128))
    negid = (np.eye(128) * NEG).astype(ml_dtypes.bfloat16)
    stepd = (c < r).astype(ml_dtypes.bfloat16)

    # per-head de-interleave: rows [even dims, odd dims]
    perm = np.concatenate([np.arange(0, HD, 2), np.arange(1, HD, 2)])

    # batch transposes computed once, shared by the 4 cores of each batch
    xts = [np.ascontiguousarray(x[b_].T) for b_ in range(B)]       # [E, T]
    onesd = np.ones((128, 64), dtype=np.float32)
    in_maps = []
    for core in range(NCORES):
        b_, hg = divmod(core, 4)
        heads = np.arange(4 * hg, 4 * hg + 4)
        rows = np.concatenate([h * HD + perm for h in heads])      # permuted
        rows_plain = np.concatenate([h * HD + np.arange(HD) for h in heads])
        # fancy-index on the transposed views: one contiguous copy each
        wqt_ = wq.T[:, rows] * scale                               # [E, DL]
        wkt_ = np.ascontiguousarray(wk.T[:, rows])
        wvt_ = np.ascontiguousarray(wv.T[:, rows_plain])
        wot_ = np.ascontiguousarray(wo.T[rows_plain, :])           # [DL, E]
        in_maps.append({
            "xt": xts[b_], "wqt": wqt_, "wkt": wkt_, "wvt": wvt_,
            "wot": wot_, "cd": C, "sd": S,
            "negid": negid, "stepd": stepd, "onesd": onesd,
        })
    return in_maps


_NC_CACHE = {}


def _get_module():
    if "nc" not in _NC_CACHE:
        _NC_CACHE["nc"] = build_module()
    return _NC_CACHE["nc"]


def _get_runner(key="nc", builder=None):
    """Build (once) a cached jax.jit shard_map callable over the 8 cores."""
    rkey = "runner_" + key
    if rkey in _NC_CACHE:
        return _NC_CACHE[rkey]
    import jax
    import concourse.mybir as _mb
    from concourse import bass2jax as b2j
    from jax.sharding import Mesh, PartitionSpec
    from jax.experimental.shard_map import shard_map

    if key == "nc":
        nc = _get_module()
    else:
        if key not in _NC_CACHE:
            _NC_CACHE[key] = builder()
        nc = _NC_CACHE[key]
    b2j.install_neuronx_cc_hook()
    partition_name = (nc.partition_id_tensor.name
                      if nc.partition_id_tensor else None)
    in_names, out_names, out_avals, zero_outs = [], [], [], []
    for alloc in nc.m.functions[0].allocations:
        if not isinstance(alloc, _mb.MemoryLocationSet):
            continue
        name = alloc.memorylocations[0].name
        if alloc.kind == "ExternalInput":
            if name != partition_name:
                in_names.append(name)
        elif alloc.kind == "ExternalOutput":
            out_names.append(name)
            shape = tuple(alloc.tensor_shape)
            dtype = _mb.dt.np(alloc.dtype)
            out_avals.append(jax.core.ShapedArray(shape, dtype))
            zero_outs.append(np.zeros(shape, dtype))
    n_params = len(in_names)
    all_names = list(in_names) + list(out_names)
    if partition_name is not None:
        all_names.append(partition_name)

    def _body(*args):
        operands = list(args)
        if partition_name is not None:
            operands.append(b2j.partition_id_tensor())
        outs = b2j._bass_exec_p.bind(
            *operands,
            out_avals=tuple(out_avals),
            in_names=tuple(all_names),
            out_names=tuple(out_names),
            lowering_input_output_aliases=(),
            sim_require_finite=True,
            sim_require_nnan=True,
            nc=nc,
        )
        return tuple(outs)

    devices = jax.devices()[:NCORES]
    mesh = Mesh(np.asarray(devices), ("core",))
    n_outs = len(out_names)
    in_specs = (PartitionSpec("core"),) * (n_params + n_outs)
    out_specs = (PartitionSpec("core"),) * n_outs
    sharded = jax.jit(
        shard_map(_body, mesh=mesh, in_specs=in_specs, out_specs=out_specs,
                  check_rep=False),
        keep_unused=True)
    from jax.sharding import NamedSharding
    _shard = NamedSharding(mesh, PartitionSpec("core"))
    concat_zeros = [
        jax.device_put(
            np.zeros((NCORES * z.shape[0], *z.shape[1:]), z.dtype), _shard)
        for z in zero_outs
    ]
    runner = {
        "sharded": sharded, "in_names": in_names, "out_names": out_names,
        "out_avals": out_avals, "concat_zeros": concat_zeros,
    }
    _NC_CACHE[rkey] = runner
    return runner


_CONST_NAMES = {"cd", "sd", "negid", "stepd", "onesd"}


def _run_spmd_cached(in_maps):
    import jax
    r = _get_runner()
    ckey = "const_dev"
    if ckey not in _NC_CACHE:
        _NC_CACHE[ckey] = {}
    const_dev = _NC_CACHE[ckey]
    concat_in = []
    for nm in r["in_names"]:
        if nm in _CONST_NAMES:
            # identical across cores and across calls: transfer once
            if nm not in const_dev:
                arr = np.concatenate(
                    [np.asarray(in_maps[c][nm]) for c in range(NCORES)],
                    axis=0)
                const_dev[nm] = jax.device_put(arr)
            concat_in.append(const_dev[nm])
        else:
            concat_in.append(np.concatenate(
                [np.asarray(in_maps[c][nm]) for c in range(NCORES)], axis=0))
    out_arrs = r["sharded"](*concat_in, *r["concat_zeros"])
    nm = r["out_names"]
    av = r["out_avals"]
    return [
        {nm[i]: np.asarray(out_arrs[i]).reshape(NCORES, *av[i].shape)[c]
         for i in range(len(nm))}
        for c in range(NCORES)
    ]


def _build_trivial():
    nc = bacc.Bacc("TRN2", target_bir_lowering=False, debug=False,
                   num_devices=NCORES)
    a = nc.dram_tensor("a", [128, 128], F32, kind="ExternalInput").ap()
    b_ = nc.dram_tensor("b", [128, 128], F32, kind="ExternalOutput").ap()
    with tile.TileContext(nc) as tc:
        with tc.tile_pool(name="t", bufs=1) as p:
            t_ = p.tile([128, 128], F32, tag="t")
            nc.sync.dma_start(out=t_[:], in_=a[:])
            nc.sync.dma_start(out=b_[:], in_=t_[:])
    nc.compile()
    return nc


def bench_hw(x, wq, wk, wv, wo, reps=9, n=30):
    """HW per-iteration time from slope: module with body repeated `reps`
    times vs once, both on the same dispatch floor."""
    import time
    import jax
    from jax.sharding import Mesh, NamedSharding, PartitionSpec

    mesh = Mesh(np.asarray(jax.devices()[:NCORES]), ("core",))
    shard = NamedSharding(mesh, PartitionSpec("core"))

    def timed(runner, concat_in):
        f = runner["sharded"]
        zs = runner["concat_zeros"]
        out = f(*concat_in, *zs)
        out[0].block_until_ready()
        ts = []
        for _ in range(n):
            t0 = time.perf_counter()
            o = f(*concat_in, *zs)
            o[0].block_until_ready()
            ts.append(time.perf_counter() - t0)
        ts.sort()
        return ts[0], ts[len(ts) // 2]

    in_maps = _prep_core_inputs(x, wq, wk, wv, wo)

    def concat(runner):
        return [
            jax.device_put(np.concatenate(
                [np.asarray(in_maps[c][nm]) for c in range(NCORES)], axis=0),
                shard)
            for nm in runner["in_names"]
        ]

    r1 = _get_runner()
    t1_min, t1_med = timed(r1, concat(r1))
    rR = _get_runner(f"nc_r{reps}", lambda: build_module(reps=reps))
    tR_min, tR_med = timed(rR, concat(rR))
    per_min = (tR_min - t1_min) / (reps - 1)
    per_med = (tR_med - t1_med) / (reps - 1)
    print(f"  x1: min {t1_min*1e3:.3f} med {t1_med*1e3:.3f} ms ; "
          f"x{reps}: min {tR_min*1e3:.3f} med {tR_med*1e3:.3f} ms")
    print(f"HW exec time: {per_min*1e9:.0f} ns (min)  {per_med*1e9:.0f} ns (med)")
    return per_min


def kernel(x, wq, wk, wv, wo, _trace=False, _trace_kwargs=None):
    x = np.asarray(x, dtype=np.float32)
    wq = np.asarray(wq, dtype=np.float32)
    wk = np.asarray(wk, dtype=np.float32)
    wv = np.asarray(wv, dtype=np.float32)
    wo = np.asarray(wo, dtype=np.float32)

    in_maps = _prep_core_inputs(x, wq, wk, wv, wo)
    try:
        results = _run_spmd_cached(in_maps)
    except Exception:
        nc = _get_module()
        results = run_bass_kernel_spmd(
            nc, in_maps, core_ids=list(range(NCORES))).results
    out = np.empty((B, T, E), dtype=np.float32)
    for b_ in range(B):
        acc = np.zeros((E, T), dtype=np.float32)
        for g in range(4):
            acc += results[4 * b_ + g]["yt"]
        out[b_] = acc.T
    return out


if __name__ == "__main__":
    nc = _get_module()
    print("module built ok")



# revision 45
# speedup vs baseline: 1.3780x; 1.3780x over previous
"""Multi-head self-attention (16 heads, hd=64, RoPE, causal) on 8 trn2 cores.

Sharding: DP(batch=2) x TP(head-groups=4). Core c handles batch c//4, heads
[4*(c%4), 4*(c%4)+4). Each core computes a row-parallel partial output
yT_partial [1024, 2048]; host sums the 4 partials per batch and transposes.
No device-device communication.

v3 schedule:
  - All matmul inputs bf16 (x, wq/wk/wv, wo host-converted); psum stays f32.
  - Phase-1 PSUM is a single 2-bank ring (tag-major q0,q1,k0,k1,v0..v3 per
    512-col half), so attention pools (scores ring 4 banks + attnv accum 2)
    coexist: attention q-blocks 0,1 (which only need window-0 Q/K/V) run
    between window-0 and window-1 projections, hiding their exp work under
    window-1's PE time.
  - RoPE partner halves live 16 partitions apart -> one DVE stream_shuffle
    straight from PSUM; cos-multiply fused into the PSUM->SBUF copy.
  - yT output-projection sub-chunks are interleaved INTO later attention
    kt-loops so the PE fills the ~143ns/kt Activation lag instead of idling.
  - Activation engine runs ONLY exp (one merged 2-head instruction per kt,
    bf16 probs out); copies on Pool/DVE; softmax denom = DVE reciprocal of
    the psum ones-row + gpsimd partition_broadcast.
"""

import sys

for _p in ("/opt/trn_rl_repo",):
    if _p not in sys.path:
        sys.path.insert(0, _p)

import numpy as np

import concourse.bass as bass
import concourse.mybir as mybir
import concourse.tile as tile
from concourse import bacc
from concourse.bass_utils import run_bass_kernel_spmd

F32 = mybir.dt.float32
F32R = mybir.dt.float32r
BF16 = mybir.dt.bfloat16
AF = mybir.ActivationFunctionType

B, T, E = 2, 2048, 1024
NH, HD = 16, 64
NHL = 4          # heads per core
DL = NHL * HD    # 256 local head dims
NCORES = 8
NEG = -1e9
ROPE_BASE = 10000.0

SHUF_MASK = [(i + 16) % 32 for i in range(32)]


# ----------------------------------------------------------------- device IR
def build_module(reps=1, debug_out=False):
    nc = bacc.Bacc("TRN2", target_bir_lowering=False, debug=False,
                   num_devices=NCORES)

    xt = nc.dram_tensor("xt", [E, T], BF16, kind="ExternalInput").ap()
    wqt = nc.dram_tensor("wqt", [E, DL], BF16, kind="ExternalInput").ap()
    wkt = nc.dram_tensor("wkt", [E, DL], BF16, kind="ExternalInput").ap()
    wvt = nc.dram_tensor("wvt", [E, DL], BF16, kind="ExternalInput").ap()
    wot = nc.dram_tensor("wot", [DL, E], BF16, kind="ExternalInput").ap()
    cd = nc.dram_tensor("cd", [128, T], F32, kind="ExternalInput").ap()
    sd = nc.dram_tensor("sd", [128, T], F32, kind="ExternalInput").ap()
    negid = nc.dram_tensor("negid", [128, 128], BF16, kind="ExternalInput").ap()
    stepd = nc.dram_tensor("stepd", [128, 128], BF16, kind="ExternalInput").ap()
    yt = nc.dram_tensor("yt", [E, T], F32, kind="ExternalOutput").ap()
    dbg = None
    if debug_out:
        dbg = {
            nm: nc.dram_tensor("dbg_" + nm, shp, BF16,
                               kind="ExternalOutput").ap()
            for nm, shp in (("q0w0", [128, 1024]), ("k0w0", [128, 1024]),
                            ("v", [128, 16 * 260]), ("at0", [128, T]),
                            ("at1", [128, T]))
        }
        for nm, shp in (("den", [1, 512]), ("rc", [1, 512]),
                        ("dnb", [64, 512])):
            dbg[nm] = nc.dram_tensor("dbg_" + nm, shp, F32,
                                     kind="ExternalOutput").ap()

    with tile.TileContext(nc) as tc:
        for _ in range(reps):
            _body(tc, xt, wqt, wkt, wvt, wot, cd, sd, negid, stepd, yt, dbg)
    nc.compile()
    return nc


def _body(tc, xt, wqt, wkt, wvt, wot, cd, sd, negid, stepd, yt, dbg=None):
    nc = tc.nc
    from contextlib import ExitStack

    with ExitStack() as outer:
        po = outer.enter_context(tc.tile_pool(name="persist", bufs=1))
        wp = outer.enter_context(tc.tile_pool(name="wp", bufs=1))
        shufp = outer.enter_context(tc.tile_pool(name="shufp", bufs=4))
        expp = outer.enter_context(tc.tile_pool(name="expp", bufs=4))
        dnp = outer.enter_context(tc.tile_pool(name="dnp", bufs=2))
        yp = outer.enter_context(tc.tile_pool(name="yp", bufs=2))

        # persistent tiles: qk[(qk, pair, win)] bf16 [128, 1024]
        qk = {}
        for nm in ("q", "k"):
            for pair in range(2):
                for win in range(2):
                    t_ = po.tile([128, 1024], BF16, tag=f"{nm}{pair}w{win}",
                                 name=f"{nm}{pair}w{win}")
                    qk[(nm, pair, win)] = t_
        v_sb = po.tile([128, 16 * 260], BF16, tag="v", name="v_sb")
        wot_sb = [po.tile([128, E], BF16, tag=f"wot{p}", name=f"wot{p}")
                  for p in range(2)]
        at = [po.tile([128, T], BF16, tag=f"at{p}", name=f"at{p}")
              for p in range(2)]
        negi_sb = po.tile([128, 128], BF16, tag="negi", name="negi_sb")
        step_sb = po.tile([128, 128], BF16, tag="step", name="step_sb")
        scratch = po.tile([1, 8], F32, tag="scr", name="scratch")
        xrow = [po.tile([128, T], BF16, tag=f"xr{eo}", name=f"xrow{eo}")
                for eo in range(8)]

        w_sb = {}
        w_srcs = {"wq": wqt, "wk": wkt, "wv": wvt}
        for nm in ("wq", "wk", "wv"):
            w_sb[nm] = wp.tile([128, 2048], BF16, tag=nm, name=nm)
        trig_c = wp.tile([128, T], F32, tag="c", name="trig_c")
        trig_s = wp.tile([128, T], F32, tag="s", name="trig_s")

        def _wload_eo(nm, eo, eng=None):
            # 2-eo chunk: w_srcs[nm][eo*128:(eo+2)*128, :] in one DMA
            (eng or nc.sync).dma_start(
                out=w_sb[nm][:, eo * 256:(eo + 2) * 256]
                .rearrange("p (e d) -> p e d", e=2),
                in_=w_srcs[nm][eo * 128:(eo + 2) * 128, :]
                .rearrange("(e p) d -> p e d", p=128))

        def _trig_load(tw):
            nc.sync.dma_start(out=trig_c[:, tw:tw + 512],
                              in_=cd[:, tw:tw + 512])
            nc.sync.dma_start(out=trig_s[:, tw:tw + 512],
                              in_=sd[:, tw:tw + 512])

        # preload the Exp activation table while the PE warms up
        nc.gpsimd.memset(scratch[0:1, :], 0.0)
        nc.scalar.activation(scratch[0:1, :], scratch[0:1, :], AF.Exp)

        # attention pools must coexist with the 2-bank phase-1 ring
        with tc.tile_pool(name="ps_s", bufs=2, space="PSUM") as ps_sp, \
             tc.tile_pool(name="ps_a", bufs=2, space="PSUM") as ps_ap:

            ytv = yt.rearrange("(et p) t -> p et t", p=128)
            pending = []          # deferred yT sub-chunk closures

            def drain_one():
                if pending:
                    pending.pop(0)()

            # ---------------- phase 1 blocks --------------------------------
            def xload(eo, win, eng=None):
                a = win * 1024
                (eng or nc.sync).dma_start(
                    out=xrow[eo][:, a:a + 1024],
                    in_=xt[eo * 128:(eo + 1) * 128, a:a + 1024])

            def prologue_dmas():
                # window-0 x columns: eo0-3 on the SP (HWDGE) queue
                # interleaved with weights, eo4-7 on the Pool (SWDGE) queue
                v_ones_view = v_sb[:].rearrange("p (tt h x) -> p tt h x",
                                                tt=16, h=4)
                nc.gpsimd.memset(v_ones_view[:, :, :, 64:65], 1.0)
                for eo in (4, 5, 6, 7):
                    xload(eo, 0, nc.gpsimd)
                _wload_eo("wq", 0)
                xload(0, 0)
                _wload_eo("wq", 2)
                xload(1, 0)
                _wload_eo("wq", 4)
                xload(2, 0)
                _wload_eo("wq", 6)
                xload(3, 0)
                _wload_eo("wk", 0)
                _wload_eo("wk", 2)
                _trig_load(0)
                _wload_eo("wk", 4)
                _wload_eo("wk", 6)
                _wload_eo("wv", 0)
                _wload_eo("wv", 2)
                nc.sync.dma_start(out=negi_sb[:], in_=negid[:])
                nc.sync.dma_start(out=step_sb[:], in_=stepd[:])
                _wload_eo("wv", 4)
                _wload_eo("wv", 6)
                _trig_load(512)

            def window1_dmas():
                # issued during the qb0/qb1 attention stretch
                for eo in range(8):
                    xload(eo, 1)
                _trig_load(1024)
                _trig_load(1536)
                for p in range(2):
                    nc.sync.dma_start(out=wot_sb[p][:],
                                      in_=wot[p * 128:(p + 1) * 128, :])

            def half_block(pp, win, half, fast=False):
                c0 = half * 512
                tw = win * 1024 + c0

                def rope(p_, nm, pair):
                    # all on DVE: GPSIMD cannot touch PSUM
                    dst = qk[(nm, pair, win)]
                    cs = slice(c0, c0 + 512)
                    shuf = shufp.tile([128, 512], F32, tag="shuf",
                                      name="shuf")
                    nc.vector.stream_shuffle(shuf[:], p_[:], SHUF_MASK)
                    nc.vector.tensor_mul(dst[:, cs], p_[:],
                                         trig_c[:, tw:tw + 512])
                    nc.vector.tensor_mul(shuf[:], shuf[:],
                                         trig_s[:, tw:tw + 512])
                    nc.vector.tensor_add(dst[:, cs], dst[:, cs], shuf[:])

                def qk_mm(p_, wnm, pair, eo):
                    nc.tensor.matmul(
                        out=p_[:],
                        lhsT=w_sb[wnm][:, eo * 256 + pair * 128:
                                       eo * 256 + pair * 128 + 128],
                        rhs=xrow[eo][:, tw:tw + 512],
                        start=(eo == 0), stop=(eo == 7))

                def v_block(tt_, psv):
                    for eo in range(8):
                        nc.tensor.matmul(
                            out=psv[:],
                            lhsT=xrow[eo][:, tw + tt_ * 128:
                                          tw + tt_ * 128 + 128],
                            rhs=w_sb["wv"][:, eo * 256:(eo + 1) * 256],
                            start=(eo == 0), stop=(eo == 7))
                    gt = tw // 128 + tt_
                    dst = v_sb[:, gt * 260:(gt + 1) * 260] \
                        .rearrange("p (h x) -> p h x", h=4)
                    src_ = psv[:].rearrange("p (h x) -> p h x", h=4)
                    # Act engine: idle during phase 1, and Pool can't
                    # read PSUM
                    nc.scalar.copy(dst[:, :, 0:64], src_[:, :, 0:64])

                if fast:
                    # first half-block: borrow the (still idle) attention
                    # psum slots so q0/q1 and k0/k1 run pair-interleaved
                    # with no rope ring-waits while x streams in
                    ps = {}
                    for pair in range(2):
                        ps[("q", pair)] = ps_sp.tile(
                            [128, 512], F32, tag="s", name="ps_qb")
                    for eo in range(8):
                        for pair in range(2):
                            qk_mm(ps[("q", pair)], "wq", pair, eo)
                    for pair in range(2):
                        rope(ps[("q", pair)], "q", pair)
                    for pair in range(2):
                        ps[("k", pair)] = pp.tile(
                            [128, 512], F32, tag="p1", name="ps_kb")
                    for eo in range(8):
                        for pair in range(2):
                            qk_mm(ps[("k", pair)], "wk", pair, eo)
                    for pair in range(2):
                        rope(ps[("k", pair)], "k", pair)
                    for tt_ in range(4):
                        if tt_ < 2:
                            psv = ps_ap.tile([128, 256], F32, tag="a",
                                             name="psv_b")
                        else:
                            psv = pp.tile([128, 256], F32, tag="p1",
                                          name="psv")
                        v_block(tt_, psv)
                    return

                # tag-major through the 2-bank ring so each pair's rope
                # overlaps the next pair's matmuls
                for wnm, pair in (("wq", 0), ("wq", 1), ("wk", 0), ("wk", 1)):
                    nm = "q" if wnm == "wq" else "k"
                    p_ = pp.tile([128, 512], F32, tag="p1",
                                 name="ps_" + nm + str(pair))
                    for eo in range(8):
                        qk_mm(p_, wnm, pair, eo)
                    rope(p_, nm, pair)

                for tt_ in range(4):
                    psv = pp.tile([128, 256], F32, tag="p1", name="psv")
                    v_block(tt_, psv)

            # ---------------- attention -------------------------------------
            def attention(pair, qb):
                # q cols [512*qb, 512*qb+512)
                qwin = qb // 2
                qoff = (qb % 2) * 512
                nkt = 4 * qb + 4
                qt = qk[("q", pair, qwin)]
                ps_a = [ps_ap.tile([128, 512], F32, tag="a", name="ps_a")
                        for _ in range(2)]
                exp_tiles = [None] * nkt
                qs_l = [0] * nkt

                def scores_step(kt):
                    kwin, koff = kt // 8, (kt % 8) * 128
                    ktile = qk[("k", pair, kwin)]
                    qs = max(0, 128 * kt - 512 * qb)
                    qs_l[kt] = qs
                    diag = kt >= 4 * qb
                    ps_s = ps_sp.tile([128, 1024], F32, tag="s", name="ps_s")
                    for h in range(2):
                        nc.tensor.matmul(
                            out=ps_s[:, 512 * h + qs:512 * h + 512],
                            lhsT=ktile[h * 64:h * 64 + 64, koff:koff + 128],
                            rhs=qt[h * 64:h * 64 + 64, qoff + qs:qoff + 512],
                            start=True, stop=not diag,
                            tile_position=(h * 64, 0))
                    if diag:
                        for h in range(2):
                            nc.tensor.matmul(
                                out=ps_s[:, 512 * h + qs:512 * h + qs + 128],
                                lhsT=negi_sb[:],
                                rhs=step_sb[:],
                                start=False, stop=True)
                    e_ = expp.tile([128, 1024], BF16, tag="e", name="exp_t")
                    ev = e_[:].rearrange("p (h c) -> p h c", h=2)
                    sv = ps_s[:].rearrange("p (h c) -> p h c", h=2)
                    nc.scalar.activation(ev[:, :, qs:], sv[:, :, qs:], AF.Exp)
                    exp_tiles[kt] = e_

                def attnv_step(kt):
                    e_ = exp_tiles[kt]
                    ev = e_[:].rearrange("p (h c) -> p h c", h=2)
                    qs = qs_l[kt]
                    for h in range(2):
                        slot = kt * 260 + (2 * pair + h) * 65
                        nc.tensor.matmul(
                            out=ps_a[h][0:65, qs:512],
                            lhsT=v_sb[:, slot:slot + 65],
                            rhs=ev[:, h, qs:512],
                            start=(kt == 0), stop=(kt == nkt - 1))
                    exp_tiles[kt] = None

                for step in range(nkt + 1):
                    if step < nkt:
                        scores_step(step)
                    if step > 0:
                        attnv_step(step - 1)
                        if step % 2 == 1:
                            drain_one()

                # normalization: reciprocal of the psum denom row (DVE ISA),
                # broadcast on Pool (SBUF-only engine), then one DVE multiply
                # per head. h1 first -- its at-rows travel through a small
                # DMA (partition shift 0->64) on the idle Activation queue.
                cs = slice(512 * qb, 512 * qb + 512)
                dh = [dnp.tile([1, 512], F32, tag=f"dh{h}", name=f"dh{h}")
                      for h in range(2)]
                rc = [dnp.tile([1, 512], F32, tag=f"rc{h}", name=f"rc{h}")
                      for h in range(2)]
                dnb = [dnp.tile([64, 512], F32, tag=f"dnb{h}",
                                name=f"dnb{h}") for h in range(2)]
                # the denom row sits at psum partition 64; the DVE recip
                # ucode needs partition-0 input, so hop through SBUF
                nc.vector.tensor_copy(dh[1][0:1, :], ps_a[1][64:65, :])
                nc.vector.tensor_copy(dh[0][0:1, :], ps_a[0][64:65, :])
                nc.vector.reciprocal_approx_fast(
                    out=rc[1][0:1, :], in_=dh[1][0:1, :])
                nc.vector.reciprocal_approx_fast(
                    out=rc[0][0:1, :], in_=dh[0][0:1, :])
                nc.gpsimd.partition_broadcast(dnb[1][:], rc[1][0:1, :])
                nc.gpsimd.partition_broadcast(dnb[0][:], rc[0][0:1, :])
                a1n = dnp.tile([64, 512], BF16, tag="a1n", name="a1n")
                nc.vector.tensor_mul(a1n[:], ps_a[1][0:64, :], dnb[1][:])
                nc.scalar.dma_start(out=at[pair][64:128, cs], in_=a1n[:])
                nc.vector.tensor_mul(at[pair][0:64, cs],
                                     ps_a[0][0:64, :], dnb[0][:])
                if dbg is not None and pair == 0 and qb == 0:
                    dend = dnp.tile([1, 512], F32, tag="dend", name="dend")
                    nc.vector.tensor_copy(dend[0:1, :], ps_a[0][64:65, :])
                    nc.sync.dma_start(out=dbg["den"][0:1, :],
                                      in_=dend[0:1, :])
                    nc.sync.dma_start(out=dbg["rc"][0:1, :],
                                      in_=rc[0][0:1, :])
                    nc.sync.dma_start(out=dbg["dnb"][:], in_=dnb[0][:])

            # ---------------- output projection sub-chunks ------------------
            def push_yt(qc, ps_yp):
                # 8 closures, one per (eg, ei) psum unit; DMA per et.
                # Copies on DVE (Pool can't read PSUM; Act is the
                # attention-phase pacer).
                state = {}

                def make(eg, ei):
                    def run():
                        et_ = eg * 4 + ei
                        if ei == 0:
                            state[eg] = yp.tile([128, 4 * 512], F32,
                                                tag="ysb", name="y_sb")
                        y_sb = state[eg]
                        ps_y = ps_yp.tile([128, 512], F32, tag="y",
                                          name="ps_y")
                        for p in range(2):
                            nc.tensor.matmul(
                                out=ps_y[:],
                                lhsT=wot_sb[p][:, et_ * 128:(et_ + 1) * 128],
                                rhs=at[p][:, qc * 512:(qc + 1) * 512],
                                start=(p == 0), stop=(p == 1))
                        nc.vector.tensor_copy(
                            y_sb[:, ei * 512:(ei + 1) * 512], ps_y[:])
                        nc.sync.dma_start(
                            out=ytv[:, et_:et_ + 1,
                                    qc * 512:(qc + 1) * 512],
                            in_=y_sb[:].rearrange(
                                "p (et t) -> p et t", et=4)[:, ei:ei + 1, :])
                    return run

                for eg in range(2):
                    for ei in range(4):
                        pending.append(make(eg, ei))

            # ---------------- main schedule ---------------------------------
            with tc.tile_pool(name="pp", bufs=2, space="PSUM") as pp:
                prologue_dmas()
                half_block(pp, 0, 0, fast=True)
                half_block(pp, 0, 1)
                # attention q-blocks 0,1 need only window-0 Q/K/V
                window1_dmas()
                attention(0, 0)
                attention(1, 0)
                attention(0, 1)
                attention(1, 1)
                half_block(pp, 1, 0)
                half_block(pp, 1, 1)

            with tc.tile_pool(name="ps_y", bufs=2, space="PSUM") as ps_yp:
                push_yt(0, ps_yp)
                push_yt(1, ps_yp)
                attention(0, 2)
                attention(1, 2)
                push_yt(2, ps_yp)
                attention(0, 3)
                attention(1, 3)
                while pending:
                    drain_one()
                # final chunk: all p0 contraction halves run during the last
                # call's exposed norm chain (their at[0] operand and the
                # borrowed psum slots are long ready); p1 halves + copies +
                # DMAs follow once at[1] lands
                qc = 3
                pools = [ps_yp, ps_sp, ps_ap]
                tags = ["y", "s", "a"]
                ps_u = []
                y_sbs = [yp.tile([128, 4 * 512], F32, tag="ysb",
                                 name="y_sb") for _ in range(2)]
                for i, (eg, ei) in enumerate(
                        (g, e) for g in range(2) for e in range(4)):
                    ps_y = pools[i % 3].tile([128, 512], F32,
                                             tag=tags[i % 3], name="ps_yf")
                    ps_u.append(ps_y)
                    nc.tensor.matmul(
                        out=ps_y[:],
                        lhsT=wot_sb[0][:, (eg * 4 + ei) * 128:
                                       (eg * 4 + ei + 1) * 128],
                        rhs=at[0][:, qc * 512:(qc + 1) * 512],
                        start=True, stop=False)
                for i, (eg, ei) in enumerate(
                        (g, e) for g in range(2) for e in range(4)):
                    et_ = eg * 4 + ei
                    ps_y = ps_u[i]
                    nc.tensor.matmul(
                        out=ps_y[:],
                        lhsT=wot_sb[1][:, et_ * 128:(et_ + 1) * 128],
                        rhs=at[1][:, qc * 512:(qc + 1) * 512],
                        start=False, stop=True)
                    y_sb = y_sbs[eg]
                    if i % 2 == 0:
                        nc.vector.tensor_copy(
                            y_sb[:, ei * 512:(ei + 1) * 512], ps_y[:])
                    else:
                        nc.scalar.copy(y_sb[:, ei * 512:(ei + 1) * 512],
                                       ps_y[:])
                    nc.sync.dma_start(
                        out=ytv[:, et_:et_ + 1, qc * 512:(qc + 1) * 512],
                        in_=y_sb[:].rearrange("p (et t) -> p et t",
                                              et=4)[:, ei:ei + 1, :])
                if dbg is not None:
                    nc.sync.dma_start(out=dbg["q0w0"][:],
                                      in_=qk[("q", 0, 0)][:])
                    nc.sync.dma_start(out=dbg["k0w0"][:],
                                      in_=qk[("k", 0, 0)][:])
                    nc.sync.dma_start(out=dbg["v"][:], in_=v_sb[:])
                    nc.sync.dma_start(out=dbg["at0"][:], in_=at[0][:])
                    nc.sync.dma_start(out=dbg["at1"][:], in_=at[1][:])


# ----------------------------------------------------------------- host side
def _prep_core_inputs(x, wq, wk, wv, wo):
    """Build the 8 per-core input dicts."""
    import ml_dtypes

    BF = ml_dtypes.bfloat16
    # rope pair layout within each 64-dim head block:
    # [x1 of pairs 0-15 | x2 of pairs 0-15 | x1 of pairs 16-31 | x2 of 16-31]
    # so the rotation partner is 16 partitions away (within a quadrant).
    perm = np.concatenate([
        np.arange(0, 32, 2), np.arange(1, 32, 2),
        np.arange(32, 64, 2), np.arange(33, 64, 2)])
    pairidx = np.concatenate([
        np.arange(16), np.arange(16), np.arange(16, 32), np.arange(16, 32)])
    sign = np.concatenate([
        -np.ones(16), np.ones(16), -np.ones(16), np.ones(16)])

    inv_freq = 1.0 / (ROPE_BASE ** (np.arange(0, HD, 2, dtype=np.float32) / HD))
    pos = np.arange(T, dtype=np.float32)
    freqs = pos[None, :] * inv_freq[:, None]           # [32, T]
    c64 = np.cos(freqs)[pairidx]                       # [64, T]
    s64 = np.sin(freqs)[pairidx] * sign[:, None]       # [64, T]
    C = np.tile(c64, (2, 1)).astype(np.float32)        # [128, T]
    S = np.tile(s64, (2, 1)).astype(np.float32)
    scale = np.float32(1.0 / np.sqrt(HD))              # folded into wq

    r, c = np.indices((128, 128))
    negid = (np.eye(128) * NEG).astype(BF)
    stepd = (c < r).astype(BF)

    xts = [np.ascontiguousarray(x[b_].T).astype(BF) for b_ in range(B)]
    in_maps = []
    for core in range(NCORES):
        b_, hg = divmod(core, 4)
        heads = np.arange(4 * hg, 4 * hg + 4)
        rows = np.concatenate([h * HD + perm for h in heads])      # permuted
        rows_plain = np.concatenate([h * HD + np.arange(HD) for h in heads])
        wqt_ = (wq.T[:, rows] * scale).astype(BF)                  # [E, DL]
        wkt_ = wk.T[:, rows].astype(BF)
        wvt_ = wv.T[:, rows_plain].astype(BF)
        wot_ = wo.T[rows_plain, :].astype(BF)                      # [DL, E]
        in_maps.append({
            "xt": xts[b_], "wqt": wqt_, "wkt": wkt_, "wvt": wvt_,
            "wot": wot_, "cd": C, "sd": S,
            "negid": negid, "stepd": stepd,
        })
    return in_maps


_NC_CACHE = {}


def _get_module():
    if "nc" not in _NC_CACHE:
        _NC_CACHE["nc"] = build_module()
    return _NC_CACHE["nc"]


def _get_runner(key="nc", builder=None):
    """Build (once) a cached jax.jit shard_map callable over the 8 cores."""
    rkey = "runner_" + key
    if rkey in _NC_CACHE:
        return _NC_CACHE[rkey]
    import jax
    import concourse.mybir as _mb
    from concourse import bass2jax as b2j
    from jax.sharding import Mesh, PartitionSpec
    from jax.experimental.shard_map import shard_map

    if key == "nc":
        nc = _get_module()
    else:
        if key not in _NC_CACHE:
            _NC_CACHE[key] = builder()
        nc = _NC_CACHE[key]
    b2j.install_neuronx_cc_hook()
    partition_name = (nc.partition_id_tensor.name
                      if nc.partition_id_tensor else None)
    in_names, out_names, out_avals, zero_outs = [], [], [], []
    for alloc in nc.m.functions[0].allocations:
        if not isinstance(alloc, _mb.MemoryLocationSet):
            continue
        name = alloc.memorylocations[0].name
        if alloc.kind == "ExternalInput":
            if name != partition_name:
                in_names.append(name)
        elif alloc.kind == "ExternalOutput":
            out_names.append(name)
            shape = tuple(alloc.tensor_shape)
            dtype = _mb.dt.np(alloc.dtype)
            out_avals.append(jax.core.ShapedArray(shape, dtype))
            zero_outs.append(np.zeros(shape, dtype))
    n_params = len(in_names)
    all_names = list(in_names) + list(out_names)
    if partition_name is not None:
        all_names.append(partition_name)

    def _body(*args):
        operands = list(args)
        if partition_name is not None:
            operands.append(b2j.partition_id_tensor())
        outs = b2j._bass_exec_p.bind(
            *operands,
            out_avals=tuple(out_avals),
            in_names=tuple(all_names),
            out_names=tuple(out_names),
            lowering_input_output_aliases=(),
            sim_require_finite=True,
            sim_require_nnan=True,
            nc=nc,
        )
        return tuple(outs)

    devices = jax.devices()[:NCORES]
    mesh = Mesh(np.asarray(devices), ("core",))
    n_outs = len(out_names)
    in_specs = (PartitionSpec("core"),) * (n_params + n_outs)
    out_specs = (PartitionSpec("core"),) * n_outs
    sharded = jax.jit(
        shard_map(_body, mesh=mesh, in_specs=in_specs, out_specs=out_specs,
                  check_rep=False),
        keep_unused=True)
    from jax.sharding import NamedSharding
    _shard = NamedSharding(mesh, PartitionSpec("core"))
    concat_zeros = [
        jax.device_put(
            np.zeros((NCORES * z.shape[0], *z.shape[1:]), z.dtype), _shard)
        for z in zero_outs
    ]
    runner = {
        "sharded": sharded, "in_names": in_names, "out_names": out_names,
        "out_avals": out_avals, "concat_zeros": concat_zeros,
    }
    _NC_CACHE[rkey] = runner
    return runner


_CONST_NAMES = {"cd", "sd", "negid", "stepd"}


def _run_spmd_cached(in_maps):
    import jax
    r = _get_runner()
    ckey = "const_dev"
    if ckey not in _NC_CACHE:
        _NC_CACHE[ckey] = {}
    const_dev = _NC_CACHE[ckey]
    concat_in = []
    for nm in r["in_names"]:
        if nm in _CONST_NAMES:
            if nm not in const_dev:
                arr = np.concatenate(
                    [np.asarray(in_maps[c][nm]) for c in range(NCORES)],
                    axis=0)
                const_dev[nm] = jax.device_put(arr)
            concat_in.append(const_dev[nm])
        else:
            concat_in.append(np.concatenate(
                [np.asarray(in_maps[c][nm]) for c in range(NCORES)], axis=0))
    out_arrs = r["sharded"](*concat_in, *r["concat_zeros"])
    nm = r["out_names"]
    av = r["out_avals"]
    return [
        {nm[i]: np.asarray(out_arrs[i]).reshape(NCORES, *av[i].shape)[c]
         for i in range(len(nm))}
        for c in range(NCORES)
    ]


def kernel(x, wq, wk, wv, wo, _trace=False, _trace_kwargs=None):
    x = np.asarray(x, dtype=np.float32)
    wq = np.asarray(wq, dtype=np.float32)
    wk = np.asarray(wk, dtype=np.float32)
    wv = np.asarray(wv, dtype=np.float32)
    wo = np.asarray(wo, dtype=np.float32)

    in_maps = _prep_core_inputs(x, wq, wk, wv, wo)
    try:
        results = _run_spmd_cached(in_maps)
    except Exception:
        nc = _get_module()
        results = run_bass_kernel_spmd(
            nc, in_maps, core_ids=list(range(NCORES))).results
    out = np.empty((B, T, E), dtype=np.float32)
    for b_ in range(B):
        acc = np.zeros((E, T), dtype=np.float32)
        for g in range(4):
            acc += results[4 * b_ + g]["yt"]
        out[b_] = acc.T
    return out


if __name__ == "__main__":
    nc = _get_module()
    print("module built ok")


# revision 46
# speedup vs baseline: 1.3831x; 1.0037x over previous
"""Multi-head self-attention (16 heads, hd=64, RoPE, causal) on 8 trn2 cores.

Sharding: DP(batch=2) x TP(head-groups=4). Core c handles batch c//4, heads
[4*(c%4), 4*(c%4)+4). Each core computes a row-parallel partial output
yT_partial [1024, 2048]; host sums the 4 partials per batch and transposes.
No device-device communication.

v3 schedule:
  - All matmul inputs bf16 (x, wq/wk/wv, wo host-converted); psum stays f32.
  - Phase-1 PSUM is a single 2-bank ring (tag-major q0,q1,k0,k1,v0..v3 per
    512-col half), so attention pools (scores ring 4 banks + attnv accum 2)
    coexist: attention q-blocks 0,1 (which only need window-0 Q/K/V) run
    between window-0 and window-1 projections, hiding their exp work under
    window-1's PE time.
  - RoPE partner halves live 16 partitions apart -> one DVE stream_shuffle
    straight from PSUM; cos-multiply fused into the PSUM->SBUF copy.
  - yT output-projection sub-chunks are interleaved INTO later attention
    kt-loops so the PE fills the ~143ns/kt Activation lag instead of idling.
  - Activation engine runs ONLY exp (one merged 2-head instruction per kt,
    bf16 probs out); copies on Pool/DVE; softmax denom = DVE reciprocal of
    the psum ones-row + gpsimd partition_broadcast.
"""

import sys

for _p in ("/opt/trn_rl_repo",):
    if _p not in sys.path:
        sys.path.insert(0, _p)

import numpy as np

import concourse.bass as bass
import concourse.mybir as mybir
import concourse.tile as tile
from concourse import bacc
from concourse.bass_utils import run_bass_kernel_spmd

F32 = mybir.dt.float32
F32R = mybir.dt.float32r
BF16 = mybir.dt.bfloat16
AF = mybir.ActivationFunctionType

B, T, E = 2, 2048, 1024
NH, HD = 16, 64
NHL = 4          # heads per core
DL = NHL * HD    # 256 local head dims
NCORES = 8
NEG = -1e9
ROPE_BASE = 10000.0

SHUF_MASK = [(i + 16) % 32 for i in range(32)]


# ----------------------------------------------------------------- device IR
def build_module(reps=1, debug_out=False):
    nc = bacc.Bacc("TRN2", target_bir_lowering=False, debug=False,
                   num_devices=NCORES)

    xt = nc.dram_tensor("xt", [E, T], BF16, kind="ExternalInput").ap()
    wqt = nc.dram_tensor("wqt", [E, DL], BF16, kind="ExternalInput").ap()
    wkt = nc.dram_tensor("wkt", [E, DL], BF16, kind="ExternalInput").ap()
    wvt = nc.dram_tensor("wvt", [E, DL], BF16, kind="ExternalInput").ap()
    wot = nc.dram_tensor("wot", [DL, E], BF16, kind="ExternalInput").ap()
    cd = nc.dram_tensor("cd", [128, T], F32, kind="ExternalInput").ap()
    sd = nc.dram_tensor("sd", [128, T], F32, kind="ExternalInput").ap()
    negid = nc.dram_tensor("negid", [128, 128], BF16, kind="ExternalInput").ap()
    stepd = nc.dram_tensor("stepd", [128, 128], BF16, kind="ExternalInput").ap()
    yt = nc.dram_tensor("yt", [E, T], BF16, kind="ExternalOutput").ap()
    dbg = None
    if debug_out:
        dbg = {
            nm: nc.dram_tensor("dbg_" + nm, shp, BF16,
                               kind="ExternalOutput").ap()
            for nm, shp in (("q0w0", [128, 1024]), ("k0w0", [128, 1024]),
                            ("v", [128, 16 * 260]), ("at0", [128, T]),
                            ("at1", [128, T]))
        }
        for nm, shp in (("den", [1, 512]), ("rc", [1, 512]),
                        ("dnb", [64, 512])):
            dbg[nm] = nc.dram_tensor("dbg_" + nm, shp, F32,
                                     kind="ExternalOutput").ap()

    with tile.TileContext(nc) as tc:
        for _ in range(reps):
            _body(tc, xt, wqt, wkt, wvt, wot, cd, sd, negid, stepd, yt, dbg)
    nc.compile()
    return nc


def _body(tc, xt, wqt, wkt, wvt, wot, cd, sd, negid, stepd, yt, dbg=None):
    nc = tc.nc
    from contextlib import ExitStack

    with ExitStack() as outer:
        po = outer.enter_context(tc.tile_pool(name="persist", bufs=1))
        wp = outer.enter_context(tc.tile_pool(name="wp", bufs=1))
        shufp = outer.enter_context(tc.tile_pool(name="shufp", bufs=4))
        expp = outer.enter_context(tc.tile_pool(name="expp", bufs=4))
        dnp = outer.enter_context(tc.tile_pool(name="dnp", bufs=2))
        yp = outer.enter_context(tc.tile_pool(name="yp", bufs=2))

        # persistent tiles: qk[(qk, pair, win)] bf16 [128, 1024]
        qk = {}
        for nm in ("q", "k"):
            for pair in range(2):
                for win in range(2):
                    t_ = po.tile([128, 1024], BF16, tag=f"{nm}{pair}w{win}",
                                 name=f"{nm}{pair}w{win}")
                    qk[(nm, pair, win)] = t_
        v_sb = po.tile([128, 16 * 260], BF16, tag="v", name="v_sb")
        wot_sb = [po.tile([128, E], BF16, tag=f"wot{p}", name=f"wot{p}")
                  for p in range(2)]
        at = [po.tile([128, T], BF16, tag=f"at{p}", name=f"at{p}")
              for p in range(2)]
        negi_sb = po.tile([128, 128], BF16, tag="negi", name="negi_sb")
        step_sb = po.tile([128, 128], BF16, tag="step", name="step_sb")
        scratch = po.tile([1, 8], F32, tag="scr", name="scratch")
        xrow = [po.tile([128, T], BF16, tag=f"xr{eo}", name=f"xrow{eo}")
                for eo in range(8)]

        w_sb = {}
        w_srcs = {"wq": wqt, "wk": wkt, "wv": wvt}
        for nm in ("wq", "wk", "wv"):
            w_sb[nm] = wp.tile([128, 2048], BF16, tag=nm, name=nm)
        trig_c = wp.tile([128, T], F32, tag="c", name="trig_c")
        trig_s = wp.tile([128, T], F32, tag="s", name="trig_s")

        def _wload_eo(nm, eo, eng=None):
            # 2-eo chunk: w_srcs[nm][eo*128:(eo+2)*128, :] in one DMA
            (eng or nc.sync).dma_start(
                out=w_sb[nm][:, eo * 256:(eo + 2) * 256]
                .rearrange("p (e d) -> p e d", e=2),
                in_=w_srcs[nm][eo * 128:(eo + 2) * 128, :]
                .rearrange("(e p) d -> p e d", p=128))

        def _trig_load(tw):
            nc.sync.dma_start(out=trig_c[:, tw:tw + 512],
                              in_=cd[:, tw:tw + 512])
            nc.sync.dma_start(out=trig_s[:, tw:tw + 512],
                              in_=sd[:, tw:tw + 512])

        # preload the Exp activation table while the PE warms up
        nc.gpsimd.memset(scratch[0:1, :], 0.0)
        nc.scalar.activation(scratch[0:1, :], scratch[0:1, :], AF.Exp)

        # attention pools must coexist with the 2-bank phase-1 ring
        with tc.tile_pool(name="ps_s", bufs=2, space="PSUM") as ps_sp, \
             tc.tile_pool(name="ps_a", bufs=2, space="PSUM") as ps_ap:

            ytv = yt.rearrange("(et p) t -> p et t", p=128)
            pending = []          # deferred yT sub-chunk closures

            def drain_one():
                if pending:
                    pending.pop(0)()

            # ---------------- phase 1 blocks --------------------------------
            def xload(eo, win, eng=None):
                a = win * 1024
                (eng or nc.sync).dma_start(
                    out=xrow[eo][:, a:a + 1024],
                    in_=xt[eo * 128:(eo + 1) * 128, a:a + 1024])

            def prologue_dmas():
                # window-0 x columns: eo0-3 on the SP (HWDGE) queue
                # interleaved with weights, eo4-7 on the Pool (SWDGE) queue
                v_ones_view = v_sb[:].rearrange("p (tt h x) -> p tt h x",
                                                tt=16, h=4)
                nc.gpsimd.memset(v_ones_view[:, :, :, 64:65], 1.0)
                for eo in (4, 5, 6, 7):
                    xload(eo, 0, nc.gpsimd)
                _wload_eo("wq", 0)
                xload(0, 0)
                _wload_eo("wq", 2)
                xload(1, 0)
                _wload_eo("wq", 4)
                xload(2, 0)
                _wload_eo("wq", 6)
                xload(3, 0)
                _wload_eo("wk", 0)
                _wload_eo("wk", 2)
                _trig_load(0)
                _wload_eo("wk", 4)
                _wload_eo("wk", 6)
                _wload_eo("wv", 0)
                _wload_eo("wv", 2)
                nc.sync.dma_start(out=negi_sb[:], in_=negid[:])
                nc.sync.dma_start(out=step_sb[:], in_=stepd[:])
                _wload_eo("wv", 4)
                _wload_eo("wv", 6)
                _trig_load(512)

            def window1_dmas():
                # issued during the qb0/qb1 attention stretch
                for eo in range(8):
                    xload(eo, 1)
                _trig_load(1024)
                _trig_load(1536)
                for p in range(2):
                    nc.sync.dma_start(out=wot_sb[p][:],
                                      in_=wot[p * 128:(p + 1) * 128, :])

            def half_block(pp, win, half, fast=False):
                c0 = half * 512
                tw = win * 1024 + c0

                def rope(p_, nm, pair):
                    # all on DVE: GPSIMD cannot touch PSUM
                    dst = qk[(nm, pair, win)]
                    cs = slice(c0, c0 + 512)
                    shuf = shufp.tile([128, 512], F32, tag="shuf",
                                      name="shuf")
                    nc.vector.stream_shuffle(shuf[:], p_[:], SHUF_MASK)
                    nc.vector.tensor_mul(dst[:, cs], p_[:],
                                         trig_c[:, tw:tw + 512])
                    nc.vector.tensor_mul(shuf[:], shuf[:],
                                         trig_s[:, tw:tw + 512])
                    nc.vector.tensor_add(dst[:, cs], dst[:, cs], shuf[:])

                def qk_mm(p_, wnm, pair, eo):
                    nc.tensor.matmul(
                        out=p_[:],
                        lhsT=w_sb[wnm][:, eo * 256 + pair * 128:
                                       eo * 256 + pair * 128 + 128],
                        rhs=xrow[eo][:, tw:tw + 512],
                        start=(eo == 0), stop=(eo == 7))

                def v_block(tt_, psv):
                    for eo in range(8):
                        nc.tensor.matmul(
                            out=psv[:],
                            lhsT=xrow[eo][:, tw + tt_ * 128:
                                          tw + tt_ * 128 + 128],
                            rhs=w_sb["wv"][:, eo * 256:(eo + 1) * 256],
                            start=(eo == 0), stop=(eo == 7))
                    gt = tw // 128 + tt_
                    dst = v_sb[:, gt * 260:(gt + 1) * 260] \
                        .rearrange("p (h x) -> p h x", h=4)
                    src_ = psv[:].rearrange("p (h x) -> p h x", h=4)
                    # Act engine: idle during phase 1, and Pool can't
                    # read PSUM
                    nc.scalar.copy(dst[:, :, 0:64], src_[:, :, 0:64])

                if fast:
                    # first half-block: borrow the (still idle) attention
                    # psum slots so q0/q1 and k0/k1 run pair-interleaved
                    # with no rope ring-waits while x streams in
                    ps = {}
                    for pair in range(2):
                        ps[("q", pair)] = ps_sp.tile(
                            [128, 512], F32, tag="s", name="ps_qb")
                    for eo in range(8):
                        for pair in range(2):
                            qk_mm(ps[("q", pair)], "wq", pair, eo)
                    for pair in range(2):
                        rope(ps[("q", pair)], "q", pair)
                    for pair in range(2):
                        ps[("k", pair)] = pp.tile(
                            [128, 512], F32, tag="p1", name="ps_kb")
                    for eo in range(8):
                        for pair in range(2):
                            qk_mm(ps[("k", pair)], "wk", pair, eo)
                    for pair in range(2):
                        rope(ps[("k", pair)], "k", pair)
                    for tt_ in range(4):
                        if tt_ < 2:
                            psv = ps_ap.tile([128, 256], F32, tag="a",
                                             name="psv_b")
                        else:
                            psv = pp.tile([128, 256], F32, tag="p1",
                                          name="psv")
                        v_block(tt_, psv)
                    return

                # tag-major through the 2-bank ring so each pair's rope
                # overlaps the next pair's matmuls
                for wnm, pair in (("wq", 0), ("wq", 1), ("wk", 0), ("wk", 1)):
                    nm = "q" if wnm == "wq" else "k"
                    p_ = pp.tile([128, 512], F32, tag="p1",
                                 name="ps_" + nm + str(pair))
                    for eo in range(8):
                        qk_mm(p_, wnm, pair, eo)
                    rope(p_, nm, pair)

                for tt_ in range(4):
                    psv = pp.tile([128, 256], F32, tag="p1", name="psv")
                    v_block(tt_, psv)

            # ---------------- attention -------------------------------------
            def attention(pair, qb):
                # q cols [512*qb, 512*qb+512)
                qwin = qb // 2
                qoff = (qb % 2) * 512
                nkt = 4 * qb + 4
                qt = qk[("q", pair, qwin)]
                ps_a = [ps_ap.tile([128, 512], F32, tag="a", name="ps_a")
                        for _ in range(2)]
                exp_tiles = [None] * nkt
                qs_l = [0] * nkt

                def scores_step(kt):
                    kwin, koff = kt // 8, (kt % 8) * 128
                    ktile = qk[("k", pair, kwin)]
                    qs = max(0, 128 * kt - 512 * qb)
                    qs_l[kt] = qs
                    diag = kt >= 4 * qb
                    ps_s = ps_sp.tile([128, 1024], F32, tag="s", name="ps_s")
                    for h in range(2):
                        nc.tensor.matmul(
                            out=ps_s[:, 512 * h + qs:512 * h + 512],
                            lhsT=ktile[h * 64:h * 64 + 64, koff:koff + 128],
                            rhs=qt[h * 64:h * 64 + 64, qoff + qs:qoff + 512],
                            start=True, stop=not diag,
                            tile_position=(h * 64, 0))
                    if diag:
                        for h in range(2):
                            nc.tensor.matmul(
                                out=ps_s[:, 512 * h + qs:512 * h + qs + 128],
                                lhsT=negi_sb[:],
                                rhs=step_sb[:],
                                start=False, stop=True)
                    e_ = expp.tile([128, 1024], BF16, tag="e", name="exp_t")
                    ev = e_[:].rearrange("p (h c) -> p h c", h=2)
                    sv = ps_s[:].rearrange("p (h c) -> p h c", h=2)
                    nc.scalar.activation(ev[:, :, qs:], sv[:, :, qs:], AF.Exp)
                    exp_tiles[kt] = e_

                def attnv_step(kt):
                    e_ = exp_tiles[kt]
                    ev = e_[:].rearrange("p (h c) -> p h c", h=2)
                    qs = qs_l[kt]
                    for h in range(2):
                        slot = kt * 260 + (2 * pair + h) * 65
                        nc.tensor.matmul(
                            out=ps_a[h][0:65, qs:512],
                            lhsT=v_sb[:, slot:slot + 65],
                            rhs=ev[:, h, qs:512],
                            start=(kt == 0), stop=(kt == nkt - 1))
                    exp_tiles[kt] = None

                for step in range(nkt + 1):
                    if step < nkt:
                        scores_step(step)
                    if step > 0:
                        attnv_step(step - 1)
                        if step % 2 == 1:
                            drain_one()

                # normalization: reciprocal of the psum denom row (DVE ISA),
                # broadcast on Pool (SBUF-only engine), then one DVE multiply
                # per head. h1 first -- its at-rows travel through a small
                # DMA (partition shift 0->64) on the idle Activation queue.
                cs = slice(512 * qb, 512 * qb + 512)
                dh = [dnp.tile([1, 512], F32, tag=f"dh{h}", name=f"dh{h}")
                      for h in range(2)]
                rc = [dnp.tile([1, 512], F32, tag=f"rc{h}", name=f"rc{h}")
                      for h in range(2)]
                dnb = [dnp.tile([64, 512], F32, tag=f"dnb{h}",
                                name=f"dnb{h}") for h in range(2)]
                # the denom row sits at psum partition 64; the DVE recip
                # ucode needs partition-0 input, so hop through SBUF
                nc.vector.tensor_copy(dh[1][0:1, :], ps_a[1][64:65, :])
                nc.vector.tensor_copy(dh[0][0:1, :], ps_a[0][64:65, :])
                nc.vector.reciprocal_approx_fast(
                    out=rc[1][0:1, :], in_=dh[1][0:1, :])
                nc.vector.reciprocal_approx_fast(
                    out=rc[0][0:1, :], in_=dh[0][0:1, :])
                nc.gpsimd.partition_broadcast(dnb[1][:], rc[1][0:1, :])
                nc.gpsimd.partition_broadcast(dnb[0][:], rc[0][0:1, :])
                a1n = dnp.tile([64, 512], BF16, tag="a1n", name="a1n")
                nc.vector.tensor_mul(a1n[:], ps_a[1][0:64, :], dnb[1][:])
                nc.scalar.dma_start(out=at[pair][64:128, cs], in_=a1n[:])
                nc.vector.tensor_mul(at[pair][0:64, cs],
                                     ps_a[0][0:64, :], dnb[0][:])
                if dbg is not None and pair == 0 and qb == 0:
                    dend = dnp.tile([1, 512], F32, tag="dend", name="dend")
                    nc.vector.tensor_copy(dend[0:1, :], ps_a[0][64:65, :])
                    nc.sync.dma_start(out=dbg["den"][0:1, :],
                                      in_=dend[0:1, :])
                    nc.sync.dma_start(out=dbg["rc"][0:1, :],
                                      in_=rc[0][0:1, :])
                    nc.sync.dma_start(out=dbg["dnb"][:], in_=dnb[0][:])

            # ---------------- output projection sub-chunks ------------------
            def push_yt(qc, ps_yp):
                # 8 closures, one per (eg, ei) psum unit; DMA per et.
                # Copies on DVE (Pool can't read PSUM; Act is the
                # attention-phase pacer).
                state = {}

                def make(eg, ei):
                    def run():
                        et_ = eg * 4 + ei
                        if ei == 0:
                            state[eg] = yp.tile([128, 4 * 512], BF16,
                                                tag="ysb", name="y_sb")
                        y_sb = state[eg]
                        ps_y = ps_yp.tile([128, 512], F32, tag="y",
                                          name="ps_y")
                        for p in range(2):
                            nc.tensor.matmul(
                                out=ps_y[:],
                                lhsT=wot_sb[p][:, et_ * 128:(et_ + 1) * 128],
                                rhs=at[p][:, qc * 512:(qc + 1) * 512],
                                start=(p == 0), stop=(p == 1))
                        nc.vector.tensor_copy(
                            y_sb[:, ei * 512:(ei + 1) * 512], ps_y[:])
                        nc.sync.dma_start(
                            out=ytv[:, et_:et_ + 1,
                                    qc * 512:(qc + 1) * 512],
                            in_=y_sb[:].rearrange(
                                "p (et t) -> p et t", et=4)[:, ei:ei + 1, :])
                    return run

                for eg in range(2):
                    for ei in range(4):
                        pending.append(make(eg, ei))

            # ---------------- main schedule ---------------------------------
            with tc.tile_pool(name="pp", bufs=2, space="PSUM") as pp:
                prologue_dmas()
                half_block(pp, 0, 0, fast=True)
                half_block(pp, 0, 1)
                # attention q-blocks 0,1 need only window-0 Q/K/V
                window1_dmas()
                attention(0, 0)
                attention(1, 0)
                attention(0, 1)
                attention(1, 1)
                half_block(pp, 1, 0)
                half_block(pp, 1, 1)

            with tc.tile_pool(name="ps_y", bufs=2, space="PSUM") as ps_yp:
                push_yt(0, ps_yp)
                push_yt(1, ps_yp)
                attention(0, 2)
                attention(1, 2)
                push_yt(2, ps_yp)
                attention(0, 3)
                attention(1, 3)
                while pending:
                    drain_one()
                # final chunk: all p0 contraction halves run during the last
                # call's exposed norm chain (their at[0] operand and the
                # borrowed psum slots are long ready); p1 halves + copies +
                # DMAs follow once at[1] lands
                qc = 3
                pools = [ps_yp, ps_sp, ps_ap]
                tags = ["y", "s", "a"]
                ps_u = []
                y_sbs = [yp.tile([128, 4 * 512], BF16, tag="ysb",
                                 name="y_sb") for _ in range(2)]
                for i, (eg, ei) in enumerate(
                        (g, e) for g in range(2) for e in range(4)):
                    ps_y = pools[i % 3].tile([128, 512], F32,
                                             tag=tags[i % 3], name="ps_yf")
                    ps_u.append(ps_y)
                    nc.tensor.matmul(
                        out=ps_y[:],
                        lhsT=wot_sb[0][:, (eg * 4 + ei) * 128:
                                       (eg * 4 + ei + 1) * 128],
                        rhs=at[0][:, qc * 512:(qc + 1) * 512],
                        start=True, stop=False)
                for i, (eg, ei) in enumerate(
                        (g, e) for g in range(2) for e in range(4)):
                    et_ = eg * 4 + ei
                    ps_y = ps_u[i]
                    nc.tensor.matmul(
                        out=ps_y[:],
                        lhsT=wot_sb[1][:, et_ * 128:(et_ + 1) * 128],
                        rhs=at[1][:, qc * 512:(qc + 1) * 512],
                        start=False, stop=True)
                    y_sb = y_sbs[eg]
                    if i % 2 == 0:
                        nc.vector.tensor_copy(
                            y_sb[:, ei * 512:(ei + 1) * 512], ps_y[:])
                    else:
                        nc.scalar.copy(y_sb[:, ei * 512:(ei + 1) * 512],
                                       ps_y[:])
                    nc.sync.dma_start(
                        out=ytv[:, et_:et_ + 1, qc * 512:(qc + 1) * 512],
                        in_=y_sb[:].rearrange("p (et t) -> p et t",
                                              et=4)[:, ei:ei + 1, :])
                if dbg is not None:
                    nc.sync.dma_start(out=dbg["q0w0"][:],
                                      in_=qk[("q", 0, 0)][:])
                    nc.sync.dma_start(out=dbg["k0w0"][:],
                                      in_=qk[("k", 0, 0)][:])
                    nc.sync.dma_start(out=dbg["v"][:], in_=v_sb[:])
                    nc.sync.dma_start(out=dbg["at0"][:], in_=at[0][:])
                    nc.sync.dma_start(out=dbg["at1"][:], in_=at[1][:])


# ----------------------------------------------------------------- host side
def _prep_core_inputs(x, wq, wk, wv, wo):
    """Build the 8 per-core input dicts."""
    import ml_dtypes

    BF = ml_dtypes.bfloat16
    # rope pair layout within each 64-dim head block:
    # [x1 of pairs 0-15 | x2 of pairs 0-15 | x1 of pairs 16-31 | x2 of 16-31]
    # so the rotation partner is 16 partitions away (within a quadrant).
    perm = np.concatenate([
        np.arange(0, 32, 2), np.arange(1, 32, 2),
        np.arange(32, 64, 2), np.arange(33, 64, 2)])
    pairidx = np.concatenate([
        np.arange(16), np.arange(16), np.arange(16, 32), np.arange(16, 32)])
    sign = np.concatenate([
        -np.ones(16), np.ones(16), -np.ones(16), np.ones(16)])

    inv_freq = 1.0 / (ROPE_BASE ** (np.arange(0, HD, 2, dtype=np.float32) / HD))
    pos = np.arange(T, dtype=np.float32)
    freqs = pos[None, :] * inv_freq[:, None]           # [32, T]
    c64 = np.cos(freqs)[pairidx]                       # [64, T]
    s64 = np.sin(freqs)[pairidx] * sign[:, None]       # [64, T]
    C = np.tile(c64, (2, 1)).astype(np.float32)        # [128, T]
    S = np.tile(s64, (2, 1)).astype(np.float32)
    scale = np.float32(1.0 / np.sqrt(HD))              # folded into wq

    r, c = np.indices((128, 128))
    negid = (np.eye(128) * NEG).astype(BF)
    stepd = (c < r).astype(BF)

    xts = [np.ascontiguousarray(x[b_].T).astype(BF) for b_ in range(B)]
    in_maps = []
    for core in range(NCORES):
        b_, hg = divmod(core, 4)
        heads = np.arange(4 * hg, 4 * hg + 4)
        rows = np.concatenate([h * HD + perm for h in heads])      # permuted
        rows_plain = np.concatenate([h * HD + np.arange(HD) for h in heads])
        wqt_ = (wq.T[:, rows] * scale).astype(BF)                  # [E, DL]
        wkt_ = wk.T[:, rows].astype(BF)
        wvt_ = wv.T[:, rows_plain].astype(BF)
        wot_ = wo.T[rows_plain, :].astype(BF)                      # [DL, E]
        in_maps.append({
            "xt": xts[b_], "wqt": wqt_, "wkt": wkt_, "wvt": wvt_,
            "wot": wot_, "cd": C, "sd": S,
            "negid": negid, "stepd": stepd,
        })
    return in_maps


_NC_CACHE = {}


def _get_module():
    if "nc" not in _NC_CACHE:
        _NC_CACHE["nc"] = build_module()
    return _NC_CACHE["nc"]


def _get_runner(key="nc", builder=None):
    """Build (once) a cached jax.jit shard_map callable over the 8 cores."""
    rkey = "runner_" + key
    if rkey in _NC_CACHE:
        return _NC_CACHE[rkey]
    import jax
    import concourse.mybir as _mb
    from concourse import bass2jax as b2j
    from jax.sharding import Mesh, PartitionSpec
    from jax.experimental.shard_map import shard_map

    if key == "nc":
        nc = _get_module()
    else:
        if key not in _NC_CACHE:
            _NC_CACHE[key] = builder()
        nc = _NC_CACHE[key]
    b2j.install_neuronx_cc_hook()
    partition_name = (nc.partition_id_tensor.name
                      if nc.partition_id_tensor else None)
    in_names, out_names, out_avals, zero_outs = [], [], [], []
    for alloc in nc.m.functions[0].allocations:
        if not isinstance(alloc, _mb.MemoryLocationSet):
            continue
        name = alloc.memorylocations[0].name
        if alloc.kind == "ExternalInput":
            if name != partition_name:
                in_names.append(name)
        elif alloc.kind == "ExternalOutput":
            out_names.append(name)
            shape = tuple(alloc.tensor_shape)
            dtype = _mb.dt.np(alloc.dtype)
            out_avals.append(jax.core.ShapedArray(shape, dtype))
            zero_outs.append(np.zeros(shape, dtype))
    n_params = len(in_names)
    all_names = list(in_names) + list(out_names)
    if partition_name is not None:
        all_names.append(partition_name)

    def _body(*args):
        operands = list(args)
        if partition_name is not None:
            operands.append(b2j.partition_id_tensor())
        outs = b2j._bass_exec_p.bind(
            *operands,
            out_avals=tuple(out_avals),
            in_names=tuple(all_names),
            out_names=tuple(out_names),
            lowering_input_output_aliases=(),
            sim_require_finite=True,
            sim_require_nnan=True,
            nc=nc,
        )
        return tuple(outs)

    devices = jax.devices()[:NCORES]
    mesh = Mesh(np.asarray(devices), ("core",))
    n_outs = len(out_names)
    in_specs = (PartitionSpec("core"),) * (n_params + n_outs)
    out_specs = (PartitionSpec("core"),) * n_outs
    sharded = jax.jit(
        shard_map(_body, mesh=mesh, in_specs=in_specs, out_specs=out_specs,
                  check_rep=False),
        keep_unused=True)
    from jax.sharding import NamedSharding
    _shard = NamedSharding(mesh, PartitionSpec("core"))
    concat_zeros = [
        jax.device_put(
            np.zeros((NCORES * z.shape[0], *z.shape[1:]), z.dtype), _shard)
        for z in zero_outs
    ]
    runner = {
        "sharded": sharded, "in_names": in_names, "out_names": out_names,
        "out_avals": out_avals, "concat_zeros": concat_zeros,
    }
    _NC_CACHE[rkey] = runner
    return runner


_CONST_NAMES = {"cd", "sd", "negid", "stepd"}


def _run_spmd_cached(in_maps):
    import jax
    r = _get_runner()
    ckey = "const_dev"
    if ckey not in _NC_CACHE:
        _NC_CACHE[ckey] = {}
    const_dev = _NC_CACHE[ckey]
    concat_in = []
    for nm in r["in_names"]:
        if nm in _CONST_NAMES:
            if nm not in const_dev:
                arr = np.concatenate(
                    [np.asarray(in_maps[c][nm]) for c in range(NCORES)],
                    axis=0)
                const_dev[nm] = jax.device_put(arr)
            concat_in.append(const_dev[nm])
        else:
            concat_in.append(np.concatenate(
                [np.asarray(in_maps[c][nm]) for c in range(NCORES)], axis=0))
    out_arrs = r["sharded"](*concat_in, *r["concat_zeros"])
    nm = r["out_names"]
    av = r["out_avals"]
    return [
        {nm[i]: np.asarray(out_arrs[i]).reshape(NCORES, *av[i].shape)[c]
         for i in range(len(nm))}
        for c in range(NCORES)
    ]


def kernel(x, wq, wk, wv, wo, _trace=False, _trace_kwargs=None):
    x = np.asarray(x, dtype=np.float32)
    wq = np.asarray(wq, dtype=np.float32)
    wk = np.asarray(wk, dtype=np.float32)
    wv = np.asarray(wv, dtype=np.float32)
    wo = np.asarray(wo, dtype=np.float32)

    in_maps = _prep_core_inputs(x, wq, wk, wv, wo)
    try:
        results = _run_spmd_cached(in_maps)
    except Exception:
        nc = _get_module()
        results = run_bass_kernel_spmd(
            nc, in_maps, core_ids=list(range(NCORES))).results
    out = np.empty((B, T, E), dtype=np.float32)
    for b_ in range(B):
        acc = np.zeros((E, T), dtype=np.float32)
        for g in range(4):
            acc += results[4 * b_ + g]["yt"].astype(np.float32)
        out[b_] = acc.T
    return out


if __name__ == "__main__":
    nc = _get_module()
    print("module built ok")
